# revision 13
# baseline (speedup 1.0000x reference)
"""Trainium2 Bass kernel for the dense branch-MLP problem.

Computes: out[b,o] = sum_n relu((s[b,:] - v[n,:]) @ W[n].T + bias[n])[o]
with B=1024, N=64, D=512, OUT=2048 in fp32.

Sharding: expert-style across the N=64 branch axis -> 8 branches per core.
Each core computes a full [B, OUT] partial sum over its 8 branches; the
host sums the 8 partials (the unshard step).

Per-core kernel (PE-bound, ~17.2 GFLOP at fp32r rates):
  - s^T resident in SBUF as 4 d-chunks [128, 1024]
  - per branch: offs = s^T - v_n (ScalarE, per-partition bias), stream
    W[n]^T tiles as matmul stationary operands, accumulate over d in PSUM,
    relu+bias on ScalarE, branch-sum on VectorE.
  - matmuls run in float32r (fp22 internal) at 1 cycle/row since the
    moving free dim is 512.
"""

import numpy as np

import concourse.bacc as bacc
import concourse.mybir as mybir
import concourse.tile as tile
from concourse.bass_utils import run_bass_kernel_spmd

B, N, D, OUT = 1024, 64, 512, 2048
N_CORES = 8
NL = N // N_CORES  # branches per core
DC = D // 128  # d chunks (4)
OT = OUT // 128  # o tiles (16)
BT = B // 512  # b free-dim tiles (2)

F32 = mybir.dt.float32
F32R = mybir.dt.float32r
RELU = mybir.ActivationFunctionType.Relu
IDENT = mybir.ActivationFunctionType.Identity

_cache = {}


def build(repeat: int = 1):
    """Build + compile the per-core Bass program. Cached per `repeat`."""
    if repeat in _cache:
        return _cache[repeat]

    nc = bacc.Bacc(
        "TRN2",
        target_bir_lowering=False,
        debug=False,
        num_devices=N_CORES,
    )

    wt_d = nc.dram_tensor("wt", [NL, 128, DC * OUT], F32R, kind="ExternalInput").ap()
    st_d = nc.dram_tensor("st", [128, DC * B], F32, kind="ExternalInput").ap()
    negv_d = nc.dram_tensor("negv", [128, NL * DC], F32, kind="ExternalInput").ap()
    bias_d = nc.dram_tensor("bias", [128, NL * OT], F32, kind="ExternalInput").ap()
    out_d = nc.dram_tensor("out", [OUT, B], F32, kind="ExternalOutput").ap()

    # o-range chunks per weight DMA: each chunk delivers o_tiles for all DC
    # d-chunks so matmul groups become ready progressively.
    WCH = 8  # wt DMA chunks per branch
    OT_PER_CH = OT // WCH

    with tile.TileContext(nc) as tc:
        with (
            tc.tile_pool(name="const", bufs=1) as const_pool,
            tc.tile_pool(name="acc", bufs=1) as acc_pool,
            tc.tile_pool(name="offs", bufs=2) as offs_pool,
            tc.tile_pool(name="wt", bufs=2) as wt_pool,
            tc.tile_pool(name="tmp", bufs=6) as tmp_pool,
            tc.tile_pool(name="psum", bufs=8, space="PSUM") as psum_pool,
        ):
            def wt_chunk_dma(wt, n, j):
                wt3 = wt[:].rearrange("p (c o) -> p c o", c=DC)
                wd3 = wt_d[n].rearrange("p (c o) -> p c o", c=DC)
                osz = OT_PER_CH * 128
                nc.sync.dma_start(
                    wt3[:, :, j * osz : (j + 1) * osz],
                    wd3[:, :, j * osz : (j + 1) * osz],
                )

            # Startup: interleave st chunks with branch-0 weight chunks so the
            # first matmul's inputs (offs c0 + wt ch0) land as early as
            # possible on the shared HBM bandwidth.
            negv = const_pool.tile([128, NL * DC], F32, name="negv")
            nc.sync.dma_start(negv[:], negv_d[:])
            st = const_pool.tile([128, DC * B], F32, name="st")
            wt0 = wt_pool.tile([128, DC * OUT], F32R, name="wt_t", tag="wt_t")
            nc.sync.dma_start(st[:, 0:B], st_d[:, 0:B])
            wt_chunk_dma(wt0, 0, 0)
            wt_chunk_dma(wt0, 0, 1)
            for c in range(1, DC):
                nc.sync.dma_start(
                    st[:, c * B : (c + 1) * B], st_d[:, c * B : (c + 1) * B]
                )
                wt_chunk_dma(wt0, 0, 1 + c)
            for j in range(1 + DC, WCH):
                wt_chunk_dma(wt0, 0, j)
            bias = const_pool.tile([128, NL * OT], F32, name="bias")
            nc.sync.dma_start(bias[:], bias_d[:])

            acc = [
                acc_pool.tile([128, B], F32, name=f"acc{ot}", tag=f"acc{ot}")
                for ot in range(OT)
            ]

            def load_wt(n):
                wt = wt_pool.tile([128, DC * OUT], F32R, name="wt_t", tag="wt_t")
                for j in range(WCH):
                    wt_chunk_dma(wt, n, j)
                return wt

            def make_offs(n):
                offs = offs_pool.tile([128, DC * B], F32R, name="offs", tag="offs")
                for c in range(DC):
                    nc.vector.tensor_scalar_add(
                        offs[:, c * B : (c + 1) * B],
                        st[:, c * B : (c + 1) * B],
                        negv[:, n * DC + c : n * DC + c + 1],
                    )
                return offs

            groups = [(ot, bt) for ot in range(OT) for bt in range(BT)]
            BATCH = 8  # interleaved psum groups (= psum banks)

            def drain_group(n, ps, ot, bt):
                b_ap = bias[:, n * OT + ot : n * OT + ot + 1]
                if n == 0:
                    nc.scalar.activation(
                        acc[ot][:, bt * 512 : bt * 512 + 512],
                        ps[:],
                        RELU,
                        bias=b_ap,
                        scale=1.0,
                    )
                else:
                    tmp = tmp_pool.tile([128, 512], F32, name="tmp", tag="tmp")
                    nc.scalar.activation(tmp[:], ps[:], RELU, bias=b_ap, scale=1.0)
                    nc.vector.tensor_add(
                        acc[ot][:, bt * 512 : bt * 512 + 512],
                        acc[ot][:, bt * 512 : bt * 512 + 512],
                        tmp[:],
                    )
                if n == NL - 1:
                    nc.sync.dma_start(
                        out_d[ot * 128 : (ot + 1) * 128, bt * 512 : bt * 512 + 512],
                        acc[ot][:, bt * 512 : bt * 512 + 512],
                    )

            def body(iv=None):
                for n in range(NL):
                    wt = wt0 if n == 0 else load_wt(n)
                    offs = make_offs(n)

                    last_branch = n == NL - 1
                    for g0 in range(0, len(groups), BATCH):
                        batch = groups[g0 : g0 + BATCH]
                        last_batch = last_branch
                        pss = [
                            psum_pool.tile([128, 512], F32, name="ps", tag="ps")
                            for _ in batch
                        ]
                        if last_batch:
                            # c-inner: groups finish one at a time so the
                            # ACT/DVE/DMA drain trickles instead of bunching
                            # after the final matmul.
                            for ps, (ot, bt) in zip(pss, batch):
                                for c in range(DC):
                                    nc.tensor.matmul(
                                        ps[:],
                                        wt[:, c * OUT + ot * 128 : c * OUT + (ot + 1) * 128],
                                        offs[:, c * B + bt * 512 : c * B + bt * 512 + 512],
                                        start=(c == 0),
                                        stop=(c == DC - 1),
                                    )
                                drain_group(n, ps, ot, bt)
                        else:
                            # d-chunk outer, group inner: PE starts as soon as
                            # the first offs/wt chunks land; later chunks
                            # stream in behind.
                            for c in range(DC):
                                for ps, (ot, bt) in zip(pss, batch):
                                    nc.tensor.matmul(
                                        ps[:],
                                        wt[:, c * OUT + ot * 128 : c * OUT + (ot + 1) * 128],
                                        offs[:, c * B + bt * 512 : c * B + bt * 512 + 512],
                                        start=(c == 0),
                                        stop=(c == DC - 1),
                                    )
                            for ps, (ot, bt) in zip(pss, batch):
                                drain_group(n, ps, ot, bt)

            if repeat == 1:
                body()
            else:
                with tc.For_i(0, repeat, 1):
                    body()

    nc.compile()
    _cache[repeat] = nc
    return nc


def prep_inputs(semantic_vec, vertices, W, b):
    """Host-side layout transforms -> per-core input maps."""
    semantic_vec = np.asarray(semantic_vec, dtype=np.float32)
    vertices = np.asarray(vertices, dtype=np.float32)
    W = np.asarray(W, dtype=np.float32)
    b = np.asarray(b, dtype=np.float32)

    # st[p, c*B + bb] = s[bb, c*128+p]
    st = np.ascontiguousarray(
        semantic_vec.reshape(B, DC, 128).transpose(2, 1, 0).reshape(128, DC * B)
    )
    # wt[n, p, c*OUT + o] = W[n, o, c*128+p]
    wt = np.ascontiguousarray(
        W.reshape(N, OUT, DC, 128).transpose(0, 3, 2, 1).reshape(N, 128, DC * OUT)
    )
    # negv[p, nl*DC + c] = -v[n0+nl, c*128+p]
    negv = np.ascontiguousarray(
        (-vertices).reshape(N_CORES, NL, DC, 128).transpose(0, 3, 1, 2).reshape(N_CORES, 128, NL * DC)
    )
    # bias[p, nl*OT + ot] = b[n0+nl, ot*128+p]
    bias = np.ascontiguousarray(
        b.reshape(N_CORES, NL, OT, 128).transpose(0, 3, 1, 2).reshape(N_CORES, 128, NL * OT)
    )

    in_maps = []
    for core in range(N_CORES):
        in_maps.append(
            {
                "wt": wt[core * NL : (core + 1) * NL],
                "st": st,
                "negv": negv[core],
                "bias": bias[core],
            }
        )
    return in_maps


def kernel(semantic_vec, vertices, W, b):
    nc = build(repeat=1)
    in_maps = prep_inputs(semantic_vec, vertices, W, b)
    res = run_bass_kernel_spmd(nc, in_maps, core_ids=list(range(N_CORES)))
    total = np.zeros((OUT, B), dtype=np.float32)
    for core in range(N_CORES):
        total += res.results[core]["out"]
    return np.ascontiguousarray(total.T)


# revision 25
# speedup vs baseline: 1.3143x; 1.3143x over previous
"""Trainium2 Bass kernel for the dense branch-MLP problem.

Computes: out[b,o] = sum_n relu((s[b,:] - v[n,:]) @ W[n].T + bias[n])[o]
with B=1024, N=64, D=512, OUT=2048 in fp32.

Sharding: expert-style across the N=64 branch axis -> 8 branches per core.
Each core computes a full [B, OUT] partial sum over its 8 branches; the
host sums the 8 partials (the unshard step).

Per-core kernel (PE-bound, ~17.2 GFLOP at fp32r rates):
  - s^T resident in SBUF as 4 d-chunks [128, 1024]
  - per branch: offs = s^T - v_n (ScalarE, per-partition bias), stream
    W[n]^T tiles as matmul stationary operands, accumulate over d in PSUM,
    relu+bias on ScalarE, branch-sum on VectorE.
  - matmuls run in float32r (fp22 internal) at 1 cycle/row since the
    moving free dim is 512.
"""

import numpy as np

import concourse.bacc as bacc
import concourse.mybir as mybir
import concourse.tile as tile
from concourse.bass_utils import run_bass_kernel_spmd

B, N, D, OUT = 1024, 64, 512, 2048
N_CORES = 8
NL = N // N_CORES  # branches per core
DC = D // 128  # d chunks (4)
OT = OUT // 128  # o tiles (16)
BT = B // 512  # b free-dim tiles (2)

F32 = mybir.dt.float32
F32R = mybir.dt.float32r
BF16 = mybir.dt.bfloat16
RELU = mybir.ActivationFunctionType.Relu
IDENT = mybir.ActivationFunctionType.Identity

_cache = {}


def build(repeat: int = 1):
    """Build + compile the per-core Bass program. Cached per `repeat`."""
    if repeat in _cache:
        return _cache[repeat]

    nc = bacc.Bacc(
        "TRN2",
        target_bir_lowering=False,
        debug=False,
        num_devices=N_CORES,
    )

    wt_d = nc.dram_tensor("wt", [NL, 128, DC * OUT], F32R, kind="ExternalInput").ap()
    st_d = nc.dram_tensor("st", [128, DC * B], F32, kind="ExternalInput").ap()
    negv_d = nc.dram_tensor("negv", [128, NL * DC], F32, kind="ExternalInput").ap()
    bias_d = nc.dram_tensor("bias", [128, NL * OT], F32, kind="ExternalInput").ap()
    out_d = nc.dram_tensor("out", [OUT, B], F32, kind="ExternalOutput").ap()

    # o-range chunks per weight DMA: each chunk delivers o_tiles for all DC
    # d-chunks so matmul groups become ready progressively.
    WCH = 8  # wt DMA chunks per branch
    OT_PER_CH = OT // WCH

    with tile.TileContext(nc) as tc:
        with (
            tc.tile_pool(name="const", bufs=1) as const_pool,
            tc.tile_pool(name="acc", bufs=1) as acc_pool,
            tc.tile_pool(name="offs", bufs=2) as offs_pool,
            tc.tile_pool(name="wt", bufs=2) as wt_pool,
            tc.tile_pool(name="tmp", bufs=6) as tmp_pool,
            tc.tile_pool(name="psum", bufs=8, space="PSUM") as psum_pool,
        ):
            def wt_chunk_dma(wt, n, j, nch=WCH):
                wt3 = wt[:].rearrange("p (c o) -> p c o", c=DC)
                wd3 = wt_d[n].rearrange("p (c o) -> p c o", c=DC)
                osz = (OT // nch) * 128
                nc.sync.dma_start(
                    wt3[:, :, j * osz : (j + 1) * osz],
                    wd3[:, :, j * osz : (j + 1) * osz],
                )

            # Startup: interleave st chunks with branch-0 weight chunks (16
            # fine chunks) so the first matmuls' inputs land as early as
            # possible on the shared HBM bandwidth and PE never outruns the
            # arrival stream.
            negv = const_pool.tile([128, NL * DC], F32, name="negv")
            nc.sync.dma_start(negv[:], negv_d[:])
            st = const_pool.tile([128, DC * B], F32, name="st")
            wt0 = wt_pool.tile([128, DC * OUT], F32R, name="wt_t", tag="wt_t")
            nc.sync.dma_start(st[:, 0:B], st_d[:, 0:B])
            wt_chunk_dma(wt0, 0, 0, nch=16)
            wt_chunk_dma(wt0, 0, 1, nch=16)
            for c in range(1, DC):
                nc.sync.dma_start(
                    st[:, c * B : (c + 1) * B], st_d[:, c * B : (c + 1) * B]
                )
                wt_chunk_dma(wt0, 0, 2 * c, nch=16)
                wt_chunk_dma(wt0, 0, 2 * c + 1, nch=16)
            for j in range(2 * DC, 16):
                wt_chunk_dma(wt0, 0, j, nch=16)
            bias = const_pool.tile([128, NL * OT], F32, name="bias")
            nc.sync.dma_start(bias[:], bias_d[:])

            acc = [
                acc_pool.tile([128, B], F32, name=f"acc{ot}", tag=f"acc{ot}")
                for ot in range(OT)
            ]

            # PE warmup: a burst of tiny matmuls on scratch data during the
            # startup DMA window, so the HAM clock gate reaches 8/8 (2.4 GHz)
            # before the first real matmul issues.
            scr = const_pool.tile([128, 128], BF16, name="scr")
            nc.vector.memset(scr[:], 0.0)
            wps = psum_pool.tile([128, 512], F32, name="wps", tag="ps")
            for _ in range(80):
                nc.tensor.matmul(
                    wps[0:64, 0:64], scr[:, 0:64], scr[:, 64:128], start=True, stop=True
                )

            def load_wt(n):
                wt = wt_pool.tile([128, DC * OUT], F32R, name="wt_t", tag="wt_t")
                for j in range(WCH):
                    wt_chunk_dma(wt, n, j)
                return wt

            def make_offs(n, dt=F32R):
                offs = offs_pool.tile([128, DC * B], dt, name="offs", tag="offs")
                for c in range(DC):
                    nc.vector.tensor_scalar_add(
                        offs[:, c * B : (c + 1) * B],
                        st[:, c * B : (c + 1) * B],
                        negv[:, n * DC + c : n * DC + c + 1],
                    )
                return offs

            groups = [(ot, bt) for ot in range(OT) for bt in range(BT)]
            BATCH = 8  # interleaved psum groups (= psum banks)

            def drain_group(n, ps, ot, bt):
                b_ap = bias[:, n * OT + ot : n * OT + ot + 1]
                if n == 0:
                    nc.scalar.activation(
                        acc[ot][:, bt * 512 : bt * 512 + 512],
                        ps[:],
                        RELU,
                        bias=b_ap,
                        scale=1.0,
                    )
                else:
                    tmp = tmp_pool.tile([128, 512], F32, name="tmp", tag="tmp")
                    nc.scalar.activation(tmp[:], ps[:], RELU, bias=b_ap, scale=1.0)
                    nc.vector.tensor_add(
                        acc[ot][:, bt * 512 : bt * 512 + 512],
                        acc[ot][:, bt * 512 : bt * 512 + 512],
                        tmp[:],
                    )
                if n == NL - 1:
                    nc.sync.dma_start(
                        out_d[ot * 128 : (ot + 1) * 128, bt * 512 : bt * 512 + 512],
                        acc[ot][:, bt * 512 : bt * 512 + 512],
                    )

            def body(iv=None):
                for n in range(NL):
                    wt = wt0 if n == 0 else load_wt(n)
                    offs = make_offs(n)

                    last_branch = n == NL - 1
                    for g0 in range(0, len(groups), BATCH):
                        batch = groups[g0 : g0 + BATCH]
                        last_batch = last_branch
                        pss = [
                            psum_pool.tile([128, 512], F32, name="ps", tag="ps")
                            for _ in batch
                        ]
                        if last_batch:
                            # c-inner: groups finish one at a time so the
                            # ACT/DVE/DMA drain trickles instead of bunching
                            # after the final matmul.
                            for ps, (ot, bt) in zip(pss, batch):
                                for c in range(DC):
                                    nc.tensor.matmul(
                                        ps[:],
                                        wt[:, c * OUT + ot * 128 : c * OUT + (ot + 1) * 128],
                                        offs[:, c * B + bt * 512 : c * B + bt * 512 + 512],
                                        start=(c == 0),
                                        stop=(c == DC - 1),
                                    )
                                drain_group(n, ps, ot, bt)
                        else:
                            # d-chunk outer, group inner: PE starts as soon as
                            # the first offs/wt chunks land; later chunks
                            # stream in behind.
                            for c in range(DC):
                                for ps, (ot, bt) in zip(pss, batch):
                                    nc.tensor.matmul(
                                        ps[:],
                                        wt[:, c * OUT + ot * 128 : c * OUT + (ot + 1) * 128],
                                        offs[:, c * B + bt * 512 : c * B + bt * 512 + 512],
                                        start=(c == 0),
                                        stop=(c == DC - 1),
                                    )
                            for ps, (ot, bt) in zip(pss, batch):
                                drain_group(n, ps, ot, bt)

            if repeat == 1:
                body()
            else:
                with tc.For_i(0, repeat, 1):
                    body()

    nc.compile()
    _cache[repeat] = nc
    return nc


def prep_inputs(semantic_vec, vertices, W, b):
    """Host-side layout transforms -> per-core input maps."""
    semantic_vec = np.asarray(semantic_vec, dtype=np.float32)
    vertices = np.asarray(vertices, dtype=np.float32)
    W = np.asarray(W, dtype=np.float32)
    b = np.asarray(b, dtype=np.float32)

    # st[p, c*B + bb] = s[bb, c*128+p]
    st = np.ascontiguousarray(
        semantic_vec.reshape(B, DC, 128).transpose(2, 1, 0).reshape(128, DC * B)
    )
    # wt[n, p, c*OUT + o] = W[n, o, c*128+p]
    wt = np.ascontiguousarray(
        W.reshape(N, OUT, DC, 128).transpose(0, 3, 2, 1).reshape(N, 128, DC * OUT)
    )
    # negv[p, nl*DC + c] = -v[n0+nl, c*128+p]
    negv = np.ascontiguousarray(
        (-vertices).reshape(N_CORES, NL, DC, 128).transpose(0, 3, 1, 2).reshape(N_CORES, 128, NL * DC)
    )
    # bias[p, nl*OT + ot] = b[n0+nl, ot*128+p]
    bias = np.ascontiguousarray(
        b.reshape(N_CORES, NL, OT, 128).transpose(0, 3, 1, 2).reshape(N_CORES, 128, NL * OT)
    )

    in_maps = []
    for core in range(N_CORES):
        in_maps.append(
            {
                "wt": wt[core * NL : (core + 1) * NL],
                "st": st,
                "negv": negv[core],
                "bias": bias[core],
            }
        )
    return in_maps


def kernel(semantic_vec, vertices, W, b):
    nc = build(repeat=1)
    in_maps = prep_inputs(semantic_vec, vertices, W, b)
    res = run_bass_kernel_spmd(nc, in_maps, core_ids=list(range(N_CORES)))
    total = np.zeros((OUT, B), dtype=np.float32)
    for core in range(N_CORES):
        total += res.results[core]["out"]
    return np.ascontiguousarray(total.T)
